# revision 7
# baseline (speedup 1.0000x reference)
"""MoE (top-2 routing, SwiGLU experts) on 8 Trainium2 NeuronCores.

Strategy: expert parallelism. Core e owns expert e's weights and computes,
for every token, that expert's SwiGLU output scaled by the token's top-2
combine weight for this expert (0 if the expert is not selected). The
per-core partial outputs (kept transposed, [D, T]) are summed across the
8 cores with per-chunk ReduceScatter collectives, so core r ends up with
rows [128r:128r+128] of out^T for each token chunk. The host reassembles
out^T and transposes back.

The router (x @ router_w, softmax, top-2 mask) is replicated on every
core in true-fp32 matmuls (top-2 selection margins require full fp32).
The expert MLP runs in bf16 (fp32 accumulation in PSUM).
"""

import numpy as np

B, S, D, E, H = 2, 2048, 1024, 8, 2048
T = B * S           # 4096 tokens
P = 128
KD = D // P         # 8  k-chunks over D
KH = H // P         # 16 k-chunks over H
NCHUNK = 8
CHUNK = T // NCHUNK  # 512 tokens per chunk
MSUB = CHUNK // P    # 4 token subtiles per chunk
NCORES = 8

# CoreSim doesn't implement the Silu activation; the sim harness flips this
# to compute silu(h) as h * sigmoid(h) instead. Hardware uses Silu directly.
USE_SILU = True

_cache = {}


def _build():
    from contextlib import ExitStack

    from concourse import bacc, mybir
    import concourse.tile as tile

    f32 = mybir.dt.float32
    bf16 = mybir.dt.bfloat16

    nc = bacc.Bacc("TRN2", target_bir_lowering=False, debug=False,
                   num_devices=NCORES)

    xT = nc.dram_tensor("xT", [D, T], f32, kind="ExternalInput")
    rw = nc.dram_tensor("rw", [D, E], f32, kind="ExternalInput")
    w1 = nc.dram_tensor("w1", [D, H], f32, kind="ExternalInput")
    w3 = nc.dram_tensor("w3", [D, H], f32, kind="ExternalInput")
    w2 = nc.dram_tensor("w2", [H, D], f32, kind="ExternalInput")
    esel = nc.dram_tensor("esel", [1, E], f32, kind="ExternalInput")
    outs = [nc.dram_tensor(f"out{c}", [P, CHUNK], f32, kind="ExternalOutput")
            for c in range(NCHUNK)]
    partials = [nc.dram_tensor(f"partial{c}", [D, CHUNK], f32)
                for c in range(NCHUNK)]
    rsouts = [nc.dram_tensor(f"rsout{c}", [P, CHUNK], f32)
              for c in range(NCHUNK)]
    combds = [nc.dram_tensor(f"combd{c}", [CHUNK], f32)
              for c in range(NCHUNK)]

    xT_v = xT.ap().rearrange("(k p) t -> p k t", p=P)      # [128, KD, T]
    w1_v = w1.ap().rearrange("(k p) h -> p k h", p=P)      # [128, KD, H]
    w3_v = w3.ap().rearrange("(k p) h -> p k h", p=P)
    w2_v = w2.ap().rearrange("(k p) d -> p k d", p=P)      # [128, KH, D]
    rw_v = rw.ap().rearrange("(k p) e -> p k e", p=P)      # [128, KD, E]

    with ExitStack() as ctx:
        tc = ctx.enter_context(tile.TileContext(nc))

        wpool = ctx.enter_context(tc.tile_pool(name="weights", bufs=1))
        stage = ctx.enter_context(tc.tile_pool(name="stage", bufs=2))
        xpool = ctx.enter_context(tc.tile_pool(name="x", bufs=1))
        xcp = ctx.enter_context(tc.tile_pool(name="xc", bufs=2))
        apool = ctx.enter_context(tc.tile_pool(name="act", bufs=1))
        ypool = ctx.enter_context(tc.tile_pool(name="y", bufs=1))
        rpool = ctx.enter_context(tc.tile_pool(name="router", bufs=2))
        psr = ctx.enter_context(tc.tile_pool(name="psr", bufs=2, space="PSUM"))
        pshg = ctx.enter_context(tc.tile_pool(name="pshg", bufs=2, space="PSUM"))
        psy = ctx.enter_context(tc.tile_pool(name="psy", bufs=2, space="PSUM"))

        # ---- persistent SBUF state ----
        rws = wpool.tile([P, KD, E], f32)
        nc.sync.dma_start(out=rws[:], in_=rw_v)
        esel_sb = wpool.tile([P, 1, E], f32)
        nc.sync.dma_start(out=esel_sb[:],
                          in_=esel.ap().partition_broadcast(P))

        def cast_copy(i, out, in_):
            # alternate DVE / ACT so fp32->bf16 casts don't serialize on one
            # engine
            if i % 2 == 0:
                nc.vector.tensor_copy(out=out, in_=in_)
            else:
                nc.scalar.copy(out=out, in_=in_)

        w1s = wpool.tile([P, KD, H], bf16)
        w3s = wpool.tile([P, KD, H], bf16)
        w2s = wpool.tile([P, KH, D], bf16)
        i = 0
        for src_v, dst, kk in ((w1_v, w1s, KD), (w3_v, w3s, KD), (w2_v, w2s, KH)):
            for k in range(kk):
                st = stage.tile([P, 2048], f32, tag="wstage")
                n = src_v.shape[2]
                nc.sync.dma_start(out=st[:, :n], in_=src_v[:, k, :])
                cast_copy(i, dst[:, k, :], st[:, :n])
                i += 1

        # ---- per-chunk pipeline ----
        for c in range(NCHUNK):
            tok = slice(c * CHUNK, (c + 1) * CHUNK)

            xst = xpool.tile([P, KD, CHUNK], f32)
            nc.sync.dma_start(out=xst[:], in_=xT_v[:, :, tok])
            xc = xcp.tile([P, KD, CHUNK], bf16)
            for k in range(KD):
                cast_copy(k, xc[:, k, :], xst[:, k, :])

            # ---- router: true-fp32 logits, token-major [128, MSUB, E] ----
            probs = rpool.tile([P, MSUB, E], f32, tag="probs")
            for m in range(MSUB):
                ps = psr.tile([P, E], f32, tag="psr")
                for k in range(KD):
                    nc.tensor.matmul(
                        out=ps[:],
                        lhsT=xst[:, k, m * P:(m + 1) * P],
                        rhs=rws[:, k, :],
                        start=(k == 0), stop=(k == KD - 1),
                    )
                # softmax numerator without max-subtraction (logits are ~N(0,1))
                nc.scalar.activation(out=probs[:, m, :], in_=ps[:],
                                     func=mybir.ActivationFunctionType.Exp)

            rsum = rpool.tile([P, MSUB, 1], f32, tag="rsum")
            nc.vector.reduce_sum(out=rsum[:], in_=probs[:],
                                 axis=mybir.AxisListType.X)
            rrec = rpool.tile([P, MSUB, 1], f32, tag="rrec")
            nc.vector.reciprocal(out=rrec[:], in_=rsum[:])
            nc.vector.tensor_mul(probs[:], probs[:],
                                 rrec[:].to_broadcast((P, MSUB, E)))
            m1 = rpool.tile([P, MSUB, 1], f32, tag="m1")
            nc.vector.reduce_max(out=m1[:], in_=probs[:],
                                 axis=mybir.AxisListType.X)
            eqm = rpool.tile([P, MSUB, E], f32, tag="eqm")
            nc.vector.tensor_tensor(out=eqm[:], in0=probs[:],
                                    in1=m1[:].to_broadcast((P, MSUB, E)),
                                    op=mybir.AluOpType.is_equal)
            masked = rpool.tile([P, MSUB, E], f32, tag="masked")
            nc.vector.tensor_scalar(out=masked[:], in0=eqm[:],
                                    scalar1=-2.0, scalar2=None,
                                    op0=mybir.AluOpType.mult)
            nc.vector.tensor_add(masked[:], masked[:], probs[:])
            m2 = rpool.tile([P, MSUB, 1], f32, tag="m2")
            nc.vector.reduce_max(out=m2[:], in_=masked[:],
                                 axis=mybir.AxisListType.X)
            gesel = rpool.tile([P, MSUB, E], f32, tag="gesel")
            nc.vector.tensor_tensor(out=gesel[:], in0=probs[:],
                                    in1=m2[:].to_broadcast((P, MSUB, E)),
                                    op=mybir.AluOpType.is_ge)
            nc.vector.tensor_mul(gesel[:], gesel[:], probs[:])
            # select this core's expert column: multiply by one-hot, reduce
            nc.vector.tensor_mul(gesel[:], gesel[:],
                                 esel_sb[:].to_broadcast((P, MSUB, E)))
            cmy = rpool.tile([P, MSUB, 1], f32, tag="cmy")
            nc.vector.reduce_sum(out=cmy[:], in_=gesel[:],
                                 axis=mybir.AxisListType.X)

            # roundtrip through DRAM to broadcast per-token weights across
            # all 128 partitions: [128, MSUB] -> dram[CHUNK] -> [128, CHUNK]
            cd = combds[c].ap().rearrange("(m p) -> p m", p=P)  # [128, MSUB]
            nc.sync.dma_start(out=cd, in_=cmy[:, :, 0])
            cbc = rpool.tile([P, CHUNK], f32, tag="cbc")
            nc.sync.dma_start(out=cbc[:],
                              in_=combds[c].ap().partition_broadcast(P))

            # ---- expert MLP: h = x@w1, g = x@w3, act = silu(h)*g ----
            act = apool.tile([P, KH, CHUNK], bf16)
            for mh in range(KH):
                psh = pshg.tile([P, CHUNK], f32, tag="psh")
                for k in range(KD):
                    nc.tensor.matmul(out=psh[:],
                                     lhsT=w1s[:, k, mh * P:(mh + 1) * P],
                                     rhs=xc[:, k, :],
                                     start=(k == 0), stop=(k == KD - 1))
                psg = pshg.tile([P, CHUNK], f32, tag="psg")
                for k in range(KD):
                    nc.tensor.matmul(out=psg[:],
                                     lhsT=w3s[:, k, mh * P:(mh + 1) * P],
                                     rhs=xc[:, k, :],
                                     start=(k == 0), stop=(k == KD - 1))
                sil = stage.tile([P, CHUNK], f32, tag="sil")
                if USE_SILU:
                    nc.scalar.activation(out=sil[:], in_=psh[:],
                                         func=mybir.ActivationFunctionType.Silu)
                    nc.vector.tensor_mul(act[:, mh, :], sil[:], psg[:])
                else:
                    nc.scalar.activation(out=sil[:], in_=psh[:],
                                         func=mybir.ActivationFunctionType.Sigmoid)
                    sil2 = stage.tile([P, CHUNK], f32, tag="sil2")
                    nc.vector.tensor_mul(sil2[:], sil[:], psg[:])
                    nc.vector.tensor_mul(act[:, mh, :], sil2[:], psh[:])

            # ---- y^T = w2^T @ act, scaled by combine weight ----
            ysb = ypool.tile([P, KD, CHUNK], f32)
            for md in range(KD):
                py = psy.tile([P, CHUNK], f32, tag="psy")
                for k in range(KH):
                    nc.tensor.matmul(out=py[:],
                                     lhsT=w2s[:, k, md * P:(md + 1) * P],
                                     rhs=act[:, k, :],
                                     start=(k == 0), stop=(k == KH - 1))
                nc.vector.tensor_mul(ysb[:, md, :], py[:], cbc[:])

            pview = partials[c].ap().rearrange("(m p) t -> p m t", p=P)
            nc.sync.dma_start(out=pview, in_=ysb[:])

            nc.gpsimd.collective_compute(
                "ReduceScatter",
                mybir.AluOpType.add,
                replica_groups=[list(range(NCORES))],
                ins=[partials[c].ap()],
                outs=[rsouts[c].ap()],
            )
            nc.sync.dma_start(out=outs[c].ap(), in_=rsouts[c].ap())

    nc.compile()
    return nc


def _get_nc():
    if "nc" not in _cache:
        _cache["nc"] = _build()
    return _cache["nc"]


def kernel(x, router_w, w1, w3, w2):
    from concourse.bass_utils import run_bass_kernel_spmd

    nc = _get_nc()

    xt = np.ascontiguousarray(x.reshape(T, D).T).astype(np.float32)
    rw = np.ascontiguousarray(router_w).astype(np.float32)
    in_maps = []
    for e in range(NCORES):
        esel = np.zeros((1, E), np.float32)
        esel[0, e] = 1.0
        in_maps.append({
            "xT": xt,
            "rw": rw,
            "w1": np.ascontiguousarray(w1[e]).astype(np.float32),
            "w3": np.ascontiguousarray(w3[e]).astype(np.float32),
            "w2": np.ascontiguousarray(w2[e]).astype(np.float32),
            "esel": esel,
        })

    res = run_bass_kernel_spmd(nc, in_maps, core_ids=list(range(NCORES)))
    _cache["last_result"] = res

    outT = np.zeros((D, T), np.float32)
    for r in range(NCORES):
        for c in range(NCHUNK):
            outT[r * P:(r + 1) * P, c * CHUNK:(c + 1) * CHUNK] = \
                res.results[r][f"out{c}"]
    return np.ascontiguousarray(outT.T).reshape(B, S, D).astype(np.float32)
